# revision 75
# baseline (speedup 1.0000x reference)
"""Trainium2 Bass kernel for a GQA causal attention block (B=2, S=2048,
HID=2048, 16 q-heads / 4 kv-heads, RoPE, causal softmax, output proj).

Sharding: core c in [0,8) handles batch b = c//4 and head-group g = c%4
(q-heads 4g..4g+3, kv-head g).  Wq/Wk/Wv are column-sharded by head group,
Wo row-sharded; each core emits a partial output and the host sums the 4
partials per batch.

Per-core kernel (all matmuls free-dim 512 where possible, bf16 inputs with
fp32 PSUM accumulation):
  - qT/kT computed in [d, s] layout directly (weights pre-transposed on
    host); RoPE applied in rotate-half form (weight rows pre-permuted
    evens-then-odds on host) via DVE ops on [64, 512] tiles.
  - scores computed TRANSPOSED, sT[k, q] = kT.T-tile @ qT, so the PV matmul
    consumes exp(sT) directly with no on-chip transposes.
  - softmax without max subtraction (scores ~N(0, 0.8); exp is safe in f32),
    denominator accumulated in f32 SBUF and reduced with a ones-matmul,
    normalization broadcast via a K=1 matmul + DVE multiply.
"""

import numpy as np
import ml_dtypes

try:
    import concourse  # noqa: F401
except ImportError:  # pragma: no cover - path fallback
    import sys

    for _p in ("/root/.axon_site/_ro/trn_rl_repo", "/opt/trn_rl_repo"):
        if _p not in sys.path:
            sys.path.append(_p)

from contextlib import ExitStack

import concourse.bass as bass
import concourse.tile as tile
from concourse import bacc, mybir
from concourse.bass_utils import run_bass_kernel_spmd

F32 = mybir.dt.float32
BF16 = mybir.dt.bfloat16

B = 2
S = 2048
HID = 2048
HEADS = 16
KV_HEADS = 4
HD = 128
HALF = HD // 2
QH = HEADS // KV_HEADS  # q heads per core (4)
LO = QH * HD  # local q/o width (512)
N_CORES = 8

NEG = -1.0e5  # additive causal mask value (exp -> exactly 0 in f32)


def _emit(ctx: ExitStack, tc: "tile.TileContext", aps: dict, s_len: int):
    nc = tc.nc
    IT = HID // 128  # contraction tiles (16)
    SC = s_len // 512  # s-chunks of 512
    KBT = s_len // 128  # 128-wide k blocks
    QBT = s_len // 512  # 512-wide q blocks

    xT, wqT, wkT, wvT, woT = aps["xT"], aps["wqT"], aps["wkT"], aps["wvT"], aps["woT"]
    chalf, shalf = aps["chalf"], aps["shalf"]
    mtri, outp = aps["mtri"], aps["outp"]

    # ---- pools ----
    xpool = ctx.enter_context(tc.tile_pool(name="xpool", bufs=4))
    spsum = ctx.enter_context(tc.tile_pool(name="spsum", bufs=4, space="PSUM"))
    ypsum = ctx.enter_context(tc.tile_pool(name="ypsum", bufs=3, space="PSUM"))
    lpsum = ctx.enter_context(tc.tile_pool(name="lpsum", bufs=1, space="PSUM"))
    ptpool = ctx.enter_context(tc.tile_pool(name="ptpool", bufs=6))
    vtpool = ctx.enter_context(tc.tile_pool(name="vtpool", bufs=2))
    ropet = ctx.enter_context(tc.tile_pool(name="ropet", bufs=4))
    bcpool = ctx.enter_context(tc.tile_pool(name="bcpool", bufs=2))
    invpool = ctx.enter_context(tc.tile_pool(name="invpool", bufs=2))
    outpool = ctx.enter_context(tc.tile_pool(name="outpool", bufs=3))

    # ---- persistent SBUF tensors ----
    def single(shape, dtype, name):
        t, free = tc.tile(shape, dtype, name=name)
        ctx.callback(free)
        return t

    wq_sb = single([128, IT, LO], BF16, "wq_sb")
    wk_sb = single([128, IT, HD], BF16, "wk_sb")
    wv_sb = single([128, IT, HD], BF16, "wv_sb")
    wo_sb = single([128, QH, HID], BF16, "wo_sb")
    ck_sb = single([128, s_len], F32, "ck_sb")  # [cos; cos]
    sk_sb = single([128, s_len], F32, "sk_sb")  # [-sin; sin]
    mtri_sb = single([128, 128], F32, "mtri_sb")
    ident_sb = single([128, 128], BF16, "ident_sb")
    qT_sb = single([128, QH, s_len], BF16, "qT_sb")
    kT_sb = single([128, s_len], BF16, "kT_sb")
    v_sb = single([128, KBT, HD], BF16, "v_sb")
    yT_sb = single([128, QH, s_len], BF16, "yT_sb")
    ones_col = single([128, 1], BF16, "ones_col")

    nc.vector.memset(ones_col, 1.0)

    # Weights on the sync DMA queue, x chunks on the gpsimd queue (parallel
    # rings) so the first matmuls start ~6us in.  wo is deferred until after
    # phase 1 so it doesn't delay startup.
    # All inputs are host-packed into the exact SBUF layout (contiguous per
    # partition), so every DMA moves maximal contiguous lines.  Startup is
    # per-ring bandwidth bound (~85GB/s each), so the chunk-0 x pieces go
    # first, striped over all three rings, then weights fill in behind.
    xs0 = xpool.tile([128, IT, 512], BF16, tag="xs", name="xs0")
    for i4 in range(IT // 4):
        eng = (nc.gpsimd, nc.sync, nc.scalar, nc.gpsimd)[i4]
        eng.dma_start(
            out=xs0[:, i4 * 4 : (i4 + 1) * 4, :], in_=xT[0, :, i4 * 4 : (i4 + 1) * 4, :]
        )
    nc.scalar.dma_start(out=wk_sb, in_=wkT)
    nc.scalar.dma_start(out=wv_sb, in_=wvT)
    for i4 in range(IT // 4):
        nc.sync.dma_start(
            out=wq_sb[:, i4 * 4 : (i4 + 1) * 4, :],
            in_=wqT[:, i4 * 4 : (i4 + 1) * 4, :],
        )
    # RoPE tables: ship f32 halves, expand + scale on device.
    # ck = [c; c], sk = [-s; s], cq/sq = scaled copies.
    nc.scalar.dma_start(out=ck_sb[0:HALF, :], in_=chalf)
    nc.scalar.dma_start(out=sk_sb[HALF:128, :], in_=shalf)
    nc.scalar.dma_start(out=ck_sb[HALF:128, :], in_=ck_sb[0:HALF, :])
    nc.scalar.dma_start(out=sk_sb[0:HALF, :], in_=sk_sb[HALF:128, :])
    nc.vector.tensor_scalar_mul(sk_sb[0:HALF, :], sk_sb[0:HALF, :], -1.0)
    # 1/sqrt(HD) is folded into the exp activation's scale operand
    qk_scale = 1.0 / float(np.sqrt(HD))
    nc.scalar.dma_start(out=mtri_sb, in_=mtri)
    nc.scalar.dma_start(out=ident_sb, in_=aps["ident"])

    # PE warm-up: ~4us of dummy matmuls on zeroed SBUF so the HAM clock gate
    # is at full rate when the first real tiles land.
    warm = ropet.tile([128, 512], F32, tag="warm", name="warm")
    nc.vector.memset(warm, 0.0)
    warm_ps = spsum.tile([128, 512], F32, tag="ps", name="warm_ps")
    wsrc = warm.bitcast(BF16)[:, 0:512]
    for wi in range(28):
        nc.tensor.matmul(
            warm_ps, wsrc[:, 0:128], wsrc, start=(wi == 0), stop=(wi == 27)
        )

    def rope(ps, out_full, cos_sb, sin_sb, sc):
        """out = RoPE(ps) in rotate-half layout; ps is a [128, 512] psum tile
        whose partitions are [evens(64); odds(64)] of one head.
        out = ps * C2 + swap_halves(ps) * S2, C2 = [cos;cos], S2 = [-sin;sin].
        """
        cs = cos_sb[:, sc * 512 : (sc + 1) * 512]
        sn = sin_sb[:, sc * 512 : (sc + 1) * 512]
        m1 = ropet.tile([128, 512], F32, tag="m1", name="m1")
        m2 = ropet.tile([128, 512], F32, tag="m2", name="m2")
        nc.vector.tensor_mul(m1, ps, cs)
        nc.vector.tensor_mul(m2[0:HALF, :], ps[HALF:128, :], sn[0:HALF, :])
        nc.vector.tensor_mul(m2[HALF:128, :], ps[0:HALF, :], sn[HALF:128, :])
        nc.vector.tensor_add(out_full, m1, m2)

    # ---------------- fused pipeline ----------------
    # One pass per 512-wide s-chunk sc: project Q/K/V for chunk sc, then run
    # attention q-block J=sc (its k-blocks only need chunks <= sc), with the
    # output projection for earlier q-blocks drip-fed from a pending queue.
    # Scores use software pipelining (LOOKAHEAD) so the PE never stalls on
    # the ACT exp chain.
    LOOKAHEAD = 3
    pending = []  # deferred finalizers / out-proj emitters (FIFO)

    def flush_one():
        if pending:
            pending.pop(0)()

    def flush_all():
        while pending:
            pending.pop(0)()

    def make_finalize(h, J, ps_y, ps_l):
        qsl = slice(J * 512, (J + 1) * 512)

        def finalize():
            l_sb = invpool.tile([1, 512], F32, tag="l_sb", name="l_sb")
            nc.vector.tensor_copy(l_sb, ps_l)
            # broadcast l across partitions (GPSIMD), then 1/l on DVE
            lbc = bcpool.tile([128, 512], F32, tag="lbc", name="lbc")
            nc.gpsimd.partition_broadcast(lbc, l_sb)
            rinv = invpool.tile([128, 512], F32, tag="rinv", name="rinv")
            nc.vector.reciprocal_approx_fast(rinv, lbc)
            nc.vector.tensor_mul(yT_sb[:, h, qsl], ps_y, rinv)

        return finalize

    def make_out(st, ob):
        def emit_out():
            ps_o = ypsum.tile([128, 512], F32, tag="ps_y", name="ps_o")
            for h in range(QH):
                nc.tensor.matmul(
                    ps_o,
                    yT_sb[:, h, st * 128 : (st + 1) * 128],
                    wo_sb[:, h, ob * 512 : (ob + 1) * 512],
                    start=(h == 0),
                    stop=(h == QH - 1),
                )
            idx = st * (HID // 512) + ob
            o_sb = outpool.tile([128, 512], BF16, tag="o_sb", name="o_sb")
            if idx % 2 == 0:
                nc.scalar.copy(o_sb, ps_o)
            else:
                nc.vector.tensor_copy(o_sb, ps_o)
            eng = (nc.sync, nc.gpsimd)[idx % 2]
            eng.dma_start(out=outp[st, ob], in_=o_sb)

        return emit_out

    for sc in range(SC):
        # ---- Q/K/V projections + RoPE for chunk sc (K first: its rope
        # unblocks this chunk's attention scores soonest) ----
        if sc == 0:
            xs = xs0
        else:
            xs = xpool.tile([128, IT, 512], BF16, tag="xs", name="xs")
            for i4 in range(IT // 4):
                eng = (nc.gpsimd, nc.sync)[(sc * 4 + i4) % 2]
                eng.dma_start(
                    out=xs[:, i4 * 4 : (i4 + 1) * 4, :],
                    in_=xT[sc, :, i4 * 4 : (i4 + 1) * 4, :],
                )

        sl = slice(sc * 512, (sc + 1) * 512)

        ps_k = spsum.tile([128, 512], F32, tag="ps", name="ps_k")
        for it in range(IT):
            nc.tensor.matmul(
                ps_k,
                wk_sb[:, it, :],
                xs[:, it, :],
                start=(it == 0),
                stop=(it == IT - 1),
            )
        rope(ps_k, kT_sb[:, sl], ck_sb, sk_sb, sc)
        flush_one()

        for h in range(QH):
            ps_q = spsum.tile([128, 512], F32, tag="ps", name="ps_q")
            for it in range(IT):
                nc.tensor.matmul(
                    ps_q,
                    wq_sb[:, it, h * HD : (h + 1) * HD],
                    xs[:, it, :],
                    start=(it == 0),
                    stop=(it == IT - 1),
                )
            rope(ps_q, qT_sb[:, h, sl], ck_sb, sk_sb, sc)
            flush_one()

        # V: compute vT[d, s] like K (N=512 matmuls, weight-load hidden),
        # then transpose 128x128 blocks on the PE into the [s, d] layout.
        ps_v = spsum.tile([128, 512], F32, tag="ps", name="ps_v")
        for it in range(IT):
            nc.tensor.matmul(
                ps_v,
                wv_sb[:, it, :],
                xs[:, it, :],
                start=(it == 0),
                stop=(it == IT - 1),
            )
        vt = vtpool.tile([128, 512], BF16, tag="vt", name="vt")
        nc.scalar.copy(vt, ps_v)
        flush_one()
        for sj in range(4):
            st = sc * 4 + sj
            ps_t = spsum.tile([128, HD], BF16, tag="ps", name="ps_t")
            nc.tensor.transpose(ps_t, vt[:, sj * 128 : (sj + 1) * 128], ident_sb)
            nc.scalar.copy(v_sb[:, st, :], ps_t)
            flush_one()

        # prefetch wo in ob-column pieces on the scalar ring, staggered so
        # piece ob arrives one chunk before the out-proj tiles that read it
        for ob in range(HID // 512):
            if min(max(0, ob - 1), SC - 1) == sc:
                nc.scalar.dma_start(
                    out=wo_sb[:, :, ob * 512 : (ob + 1) * 512],
                    in_=woT[:, :, ob * 512 : (ob + 1) * 512],
                )

        # ---- attention q-block J = sc ----
        J = sc
        nkb = 4 * J + 4
        for h in range(QH):
            ps_y = ypsum.tile([128, 512], F32, tag="ps_y", name="ps_y")
            ps_l = lpsum.tile([1, 512], F32, tag="ps_l", name="ps_l")

            rest_q = []

            def emit_rest(kb, ps_s, h=h, J=J, ps_y=ps_y, ps_l=ps_l, nkb=nkb):
                r = kb - 4 * J
                lo = r * 128 if r >= 0 else 0
                if r >= 0:
                    nc.vector.tensor_add(
                        ps_s[:, lo : lo + 128], ps_s[:, lo : lo + 128], mtri_sb
                    )
                pt = ptpool.tile([128, 512], BF16, tag="pt", name="pt")
                nc.scalar.activation(
                    pt[:, lo:512],
                    ps_s[:, lo:512],
                    mybir.ActivationFunctionType.Exp,
                    scale=qk_scale,
                )
                nc.tensor.matmul(
                    ps_y[:, lo:512],
                    v_sb[:, kb, :],
                    pt[:, lo:512],
                    start=(kb == 0),
                    stop=(kb == nkb - 1),
                )
                # softmax denominator: accumulate column sums of pt on the PE
                # (after PV: its ones weight-load is trivial, so the PE's
                # background weight buffer stays free for the next k-tile)
                nc.tensor.matmul(
                    ps_l[:, lo:512],
                    ones_col,
                    pt[:, lo:512],
                    start=(kb == 0),
                    stop=(kb == nkb - 1),
                )

            for kb in range(nkb):
                r = kb - 4 * J  # >=0 on diagonal blocks
                lo = r * 128 if r >= 0 else 0

                ps_s = spsum.tile([128, 512], F32, tag="ps", name="ps_s")
                nc.tensor.matmul(
                    ps_s[:, lo:512],
                    kT_sb[:, kb * 128 : (kb + 1) * 128],
                    qT_sb[:, h, J * 512 + lo : (J + 1) * 512],
                    start=True,
                    stop=True,
                )
                rest_q.append((kb, ps_s))
                if kb % 2 == 1:
                    flush_one()
                if len(rest_q) > LOOKAHEAD:
                    emit_rest(*rest_q.pop(0))
            while rest_q:
                emit_rest(*rest_q.pop(0))

            pending.append(make_finalize(h, J, ps_y, ps_l))

        # out-proj tiles for q-block J-1 join the queue now (one-chunk delay
        # keeps them clear of this chunk's x loads and the wo prefetch)
        if J > 0:
            for ob in range(HID // 512):
                for sj in range(4):
                    pending.append(make_out((J - 1) * 4 + sj, ob))

    for ob in range(HID // 512):
        for sj in range(4):
            pending.append(make_out((SC - 1) * 4 + sj, ob))
    flush_all()


def build_module(s_len: int = S):
    nc = bacc.Bacc(
        "TRN2", target_bir_lowering=False, debug=False, enable_asserts=False
    )
    IT = HID // 128
    SC = s_len // 512
    aps = {}
    aps["xT"] = nc.dram_tensor(
        "xT", [SC, 128, IT, 512], BF16, kind="ExternalInput"
    ).ap()
    aps["wqT"] = nc.dram_tensor("wqT", [128, IT, LO], BF16, kind="ExternalInput").ap()
    aps["wkT"] = nc.dram_tensor("wkT", [128, IT, HD], BF16, kind="ExternalInput").ap()
    aps["wvT"] = nc.dram_tensor("wvT", [128, IT, HD], BF16, kind="ExternalInput").ap()
    aps["woT"] = nc.dram_tensor("woT", [128, QH, HID], BF16, kind="ExternalInput").ap()
    aps["chalf"] = nc.dram_tensor(
        "chalf", [HALF, s_len], F32, kind="ExternalInput"
    ).ap()
    aps["shalf"] = nc.dram_tensor(
        "shalf", [HALF, s_len], F32, kind="ExternalInput"
    ).ap()
    aps["mtri"] = nc.dram_tensor("mtri", [128, 128], F32, kind="ExternalInput").ap()
    aps["ident"] = nc.dram_tensor(
        "ident", [128, 128], BF16, kind="ExternalInput"
    ).ap()
    aps["outp"] = nc.dram_tensor(
        "outp", [s_len // 128, HID // 512, 128, 512], BF16, kind="ExternalOutput"
    ).ap()

    with tile.TileContext(nc) as tc:
        with ExitStack() as ctx:
            _emit(ctx, tc, aps, s_len)
    nc.compile()
    return nc


_MODULE_CACHE: dict = {}


def _get_module(s_len: int = S):
    if s_len not in _MODULE_CACHE:
        _MODULE_CACHE[s_len] = build_module(s_len)
    return _MODULE_CACHE[s_len]


_PERM = np.concatenate([np.arange(0, HD, 2), np.arange(1, HD, 2)])  # evens|odds


def make_in_maps(x, cos, sin, Wq, Wk, Wv, Wo, s_len: int = S):
    """Build the 8 per-core input maps (host-side sharding + layout prep)."""
    x = np.asarray(x, dtype=np.float32)
    cos = np.asarray(cos, dtype=np.float32)
    sin = np.asarray(sin, dtype=np.float32)
    Wq = np.asarray(Wq, dtype=np.float32)
    Wk = np.asarray(Wk, dtype=np.float32)
    Wv = np.asarray(Wv, dtype=np.float32)
    Wo = np.asarray(Wo, dtype=np.float32)

    bf = ml_dtypes.bfloat16
    scale = 1.0 / np.sqrt(HD)

    # rotate-half table halves; device expands to [c;c], [-s;s] and scales
    chalf = np.ascontiguousarray(cos.T).astype(np.float32)  # [64, S]
    shalf = np.ascontiguousarray(sin.T).astype(np.float32)

    kk, qq = np.meshgrid(np.arange(128), np.arange(128), indexing="ij")
    mtri = np.where(kk <= qq, 0.0, NEG).astype(np.float32)

    Wq4 = Wq.reshape(HEADS, HD, HID)
    Wk4 = Wk.reshape(KV_HEADS, HD, HID)
    Wv4 = Wv.reshape(KV_HEADS, HD, HID)

    IT = HID // 128
    SC = s_len // 512

    def pack_w(w_l):  # [O, HID] -> [128(p), IT, O]; i = it*128 + p
        return np.ascontiguousarray(
            w_l.T.reshape(IT, 128, w_l.shape[0]).transpose(1, 0, 2)
        ).astype(bf)

    in_maps = []
    xT_cache = {}
    for c in range(N_CORES):
        b, g = divmod(c, KV_HEADS)
        hs = [g * QH + i for i in range(QH)]
        wq_l = Wq4[hs][:, _PERM, :].reshape(LO, HID)  # [512, 2048]
        wk_l = Wk4[g][_PERM, :]  # [128, 2048]
        wv_l = Wv4[g]  # [128, 2048]
        jcols = np.concatenate([np.arange(h * HD, (h + 1) * HD) for h in hs])
        wo_l = Wo[:, jcols]  # [2048, 512]

        if b not in xT_cache:
            # [SC, 128(p), IT, 512(s)]; x[b][sc*512+s, it*128+p]
            xT_cache[b] = np.ascontiguousarray(
                x[b]
                .reshape(SC, 512, IT, 128)
                .transpose(0, 3, 2, 1)
            ).astype(bf)

        in_maps.append(
            {
                "xT": xT_cache[b],
                "wqT": pack_w(wq_l),
                "wkT": pack_w(wk_l),
                "wvT": pack_w(wv_l),
                # [128(p), QH(jt), HID(o)]; j = jt*128 + p
                "woT": np.ascontiguousarray(
                    wo_l.T.reshape(QH, 128, HID).transpose(1, 0, 2)
                ).astype(bf),
                "chalf": chalf,
                "shalf": shalf,
                "mtri": mtri,
                "ident": np.eye(128, dtype=np.float32).astype(bf),
            }
        )
    return in_maps


def combine_outputs(results):
    out = np.zeros((B, S, HID), dtype=np.float32)
    for c in range(N_CORES):
        b = c // KV_HEADS
        # outp is [S//128, HID//512, 128, 512] device-layout (bf16 partials)
        part = (
            results[c]["outp"].astype(np.float32).transpose(0, 2, 1, 3).reshape(S, HID)
        )
        out[b] += part
    return out


def kernel(x, cos, sin, Wq, Wk, Wv, Wo):
    nc = _get_module(S)
    in_maps = make_in_maps(x, cos, sin, Wq, Wk, Wv, Wo, S)
    res = run_bass_kernel_spmd(nc, in_maps, core_ids=list(range(N_CORES)))
    return combine_outputs(res.results)


def run_traced(x, cos, sin, Wq, Wk, Wv, Wo, **trace_kwargs):
    """Like kernel() but with NTFF tracing; returns (output, BassKernelResults)."""
    nc = _get_module(S)
    in_maps = make_in_maps(x, cos, sin, Wq, Wk, Wv, Wo, S)
    res = run_bass_kernel_spmd(
        nc, in_maps, core_ids=list(range(N_CORES)), trace=True, **trace_kwargs
    )
    return combine_outputs(res.results), res
